# revision 1
# baseline (speedup 1.0000x reference)
"""CURLoRA layer kernel for 8 TRN2 NeuronCores.

Computes out = x @ (W + C@U@R)^T + bias for
  x: (4, 2048, 4096) f32, W: (4096, 4096), C: (4096, 64), U: (64, 64),
  R: (64, 4096), bias: (4096,)  ->  out: (4, 2048, 4096) f32

Sharding: 8 cores = 2 token-groups x 4 output-column-groups.
Each core computes out[tg, og] = x[tg] @ (W[og] + C[og]@U@R)^T + bias[og]
independently (no collectives needed).

Per-core kernel (bf16 compute, fp32 accumulate):
  1. Build W'^T = (W_sh + C_sh@U@R)^T in SBUF as bf16 [d=128p, 32k, 1024o].
  2. Stream x tiles [128t, 4096d], cast to bf16 (SWDGE cast-DMA),
     PE-transpose into x^T tiles [128d-part, 32k, 128t].
  3. Accumulate out[t, o] over 32 k-tiles into PSUM, add bias on eviction.
A few x tiles are transposed before/through the W'-build to keep the PE
array dense early (HAM clock-gate warmup).
"""

import sys

if "/opt/trn_rl_repo" not in sys.path:
    sys.path.insert(0, "/opt/trn_rl_repo")

import numpy as np

B, S, D = 4, 2048, 4096
O = 4096
RK = 64
T = B * S  # 8192 tokens
NT, NO = 2, 4  # token groups x out-column groups
TSH = T // NT  # 4096 tokens per core
OSH = O // NO  # 1024 out columns per core
N_CORES = 8

_CACHE = {}


def _build():
    from concourse import bacc
    import concourse.bass as bass
    import concourse.mybir as mybir
    from concourse.bass import ts
    from concourse.tile import TileContext
    from concourse.masks import make_identity

    f32 = mybir.dt.float32
    bf16 = mybir.dt.bfloat16

    nc = bacc.Bacc()
    x_ext = nc.declare_dram_parameter("x", [TSH, D], f32, isOutput=False)
    w_ext = nc.declare_dram_parameter("W", [OSH, D], f32, isOutput=False)
    c_ext = nc.declare_dram_parameter("C", [OSH, RK], f32, isOutput=False)
    u_ext = nc.declare_dram_parameter("U", [RK, RK], f32, isOutput=False)
    r_ext = nc.declare_dram_parameter("R", [RK, D], f32, isOutput=False)
    b_ext = nc.declare_dram_parameter("bias", [OSH], f32, isOutput=False)
    out_ext = nc.declare_dram_parameter("out", [TSH, OSH], f32, isOutput=True)

    NKT = D // 128  # 32 k-tiles
    NTT = TSH // 128  # 32 t-tiles per core
    NOJ = OSH // 512  # 2 o-blocks of 512
    NPRE = 3  # x tiles transposed ahead of the W' build

    with TileContext(nc) as tc:
        with (
            tc.tile_pool(name="const", bufs=1) as const,
            tc.tile_pool(name="wt", bufs=1) as wtp,
            tc.tile_pool(name="small", bufs=1) as small,
            tc.tile_pool(name="xpool", bufs=2) as xpool,
            tc.tile_pool(name="xtpool", bufs=NPRE + 2) as xtpool,
            tc.tile_pool(name="opool", bufs=3) as opool,
            tc.tile_pool(name="wpool", bufs=4) as wpool,
            # PSUM: psT (t 3x1 + s 1x1) + psA 2x1 + psW 2x1 = 8 banks
            tc.tile_pool(name="psT", bufs=3, space="PSUM") as psT,
            tc.tile_pool(name="psA", bufs=2, space="PSUM") as psA,
            tc.tile_pool(name="psW", bufs=2, space="PSUM") as psW,
        ):
            ident = const.tile([128, 128], bf16)
            make_identity(nc, ident)
            identf = const.tile([128, 128], f32)
            make_identity(nc, identf)

            # scratch target for HAM-warming dummy transposes (never read)
            warm_ps = psW.tile([128, 128], bf16, tag="w")

            def emit_pe_warm(n):
                # keep the PE HAM clock-gate warm while real PE work waits
                # on DMA/DVE: transpose identity into a scratch PSUM bank
                for i in range(n):
                    nc.tensor.transpose(warm_ps[:], ident[:], ident[:])

            # resident W'^T: [128 d-part, 32 k-tiles, 1024 o] bf16
            wt_sb = wtp.tile([128, NKT, OSH], bf16)
            w2_sb = [
                wtp.tile([128, D], bf16, name="w2a"),
                wtp.tile([128, D], bf16, name="w2b"),
            ]
            bias_sb = const.tile([128, OSH], f32)

            def emit_x_transpose(i):
                x_nat = xpool.tile([128, D], bf16)
                nc.gpsimd.dma_start(out=x_nat[:], in_=x_ext[ts(i, 128), :])
                xT = xtpool.tile([128, NKT, 128], bf16)
                for g in range(NKT // 8):
                    ps_xt = psT.tile([128, 8, 128], bf16, tag="t", bufs=3)
                    for ii in range(8):
                        nc.tensor.transpose(
                            ps_xt[:, ii, :], x_nat[:, ts(8 * g + ii, 128)], ident[:]
                        )
                    nc.vector.tensor_copy(out=xT[:, 8 * g : 8 * g + 8, :], in_=ps_xt[:])
                return xT

            def emit_mm_j(i, j, xT, out_sb):
                psm = psA.tile([128, 512], f32, tag="a")
                for k in range(NKT):
                    nc.tensor.matmul(
                        psm[:],
                        xT[:, k, :],
                        wt_sb[:, k, ts(j, 512)],
                        start=(k == 0),
                        stop=(k == NKT - 1),
                    )
                nc.vector.tensor_add(
                    out=out_sb[:, ts(j, 512)],
                    in0=psm[:],
                    in1=bias_sb[:, ts(j, 512)],
                )

            def emit_x_mm(i, xT):
                out_sb = opool.tile([128, OSH], f32, tag="out_sb")
                for j in range(NOJ):
                    emit_mm_j(i, j, xT, out_sb)
                    nc.sync.dma_start(
                        out_ext[ts(i, 128), ts(j, 512)], out_sb[:, ts(j, 512)]
                    )

            # small adapter inputs first on the SWDGE queue (they gate the
            # adapter chain; x casts are bigger and can wait ~4us)
            u_sb = small.tile([RK, RK], bf16)
            nc.gpsimd.dma_start(out=u_sb[:], in_=u_ext[:])  # cast f32->bf16
            c_nat = small.tile([128, OSH // 128, RK], bf16)
            nc.gpsimd.dma_start(
                out=c_nat[:], in_=c_ext[:].rearrange("(j p) r -> p j r", p=128)
            )
            r_sb = small.tile([RK, D], bf16)
            nc.gpsimd.dma_start(out=r_sb[:], in_=r_ext[:])
            ut_sb = small.tile([RK, RK], bf16)
            ct_sb = small.tile([RK, OSH], bf16)
            ur_sb = small.tile([RK, D], bf16)

            # --- tiny warm block while U/C DMAs land ---
            emit_pe_warm(24)

            # U^T
            ps_u = psT.tile([RK, 128], bf16, tag="s", bufs=1)
            nc.tensor.transpose(ps_u[:, :RK], u_sb[:], ident[:RK, :RK])
            nc.vector.tensor_copy(out=ut_sb[:], in_=ps_u[:, :RK])

            # C^T tiles
            for j in range(OSH // 128):
                ps_c = psT.tile([RK, 128], bf16, tag="s", bufs=1)
                nc.tensor.transpose(ps_c[:], c_nat[:, j, :], ident[:])
                nc.vector.tensor_copy(out=ct_sb[:, ts(j, 128)], in_=ps_c[:])

            emit_pe_warm(24)

            # UR = U @ R  -> [64, 4096] bf16
            for c in range(D // 512):
                ps_ur = psA.tile([128, 512], f32, tag="a")
                nc.tensor.matmul(
                    ps_ur[:RK, :], ut_sb[:], r_sb[:, ts(c, 512)], start=True, stop=True
                )
                nc.vector.tensor_copy(out=ur_sb[:, ts(c, 512)], in_=ps_ur[:RK, :])

            # --- early x tiles: dense PE work while W' build streams in ---
            pre_xt = [emit_x_transpose(i) for i in range(NPRE)]

            # W' = W + C@UR per o-tile, then transpose into wt_sb.
            # The first t-tiles' j=0 matmuls are interleaved after the first
            # half of W'^T is ready, so the PE runs main GEMM work while the
            # DVE/DMA finish the remaining W tiles.
            out_pre = [
                opool.tile([128, OSH], f32, tag="out_sb", name="out_pre")
                for _ in range(NPRE)
            ]

            def w_build_tile(j):
                w2 = w2_sb[j % 2]
                for h in range(2):
                    w_nat = wpool.tile([128, D // 2], f32, name="w_nat")
                    nc.sync.dma_start(
                        w_nat[:], w_ext[ts(j, 128), ts(h, D // 2)]
                    )
                    for ch in range(D // 1024):
                        c = h * (D // 1024) + ch
                        ps_ad = psA.tile([128, 512], f32, tag="a")
                        nc.tensor.matmul(
                            ps_ad[:],
                            ct_sb[:, ts(j, 128)],
                            ur_sb[:, ts(c, 512)],
                            start=True,
                            stop=True,
                        )
                        nc.vector.tensor_add(
                            out=w2[:, ts(c, 512)],
                            in0=ps_ad[:],
                            in1=w_nat[:, ts(ch, 512)],
                        )
                    for g in range(2):
                        gg = h * 2 + g
                        ps_wt = psW.tile([128, 8, 128], bf16, tag="w")
                        for i in range(8):
                            nc.tensor.transpose(
                                ps_wt[:, i, :], w2[:, ts(8 * gg + i, 128)], ident[:]
                            )
                        nc.scalar.copy(
                            out=wt_sb[:, 8 * gg : 8 * gg + 8, ts(j, 128)],
                            in_=ps_wt[:],
                        )

            # bias broadcast to all 128 partitions (consumed by the first
            # evictions interleaved below)
            b_ap = b_ext[:]
            b_bc = bass.AP(
                tensor=b_ap.tensor,
                offset=b_ap.offset,
                ap=[[0, 128]] + [list(p) for p in b_ap.ap],
            )
            nc.gpsimd.dma_start(out=bias_sb[:], in_=b_bc)

            for j in range(4):
                w_build_tile(j)
            emit_mm_j(0, 0, pre_xt[0], out_pre[0])
            w_build_tile(4)
            emit_mm_j(1, 0, pre_xt[1], out_pre[1])
            w_build_tile(5)
            pre_xt.append(emit_x_transpose(NPRE))
            w_build_tile(6)
            emit_mm_j(2, 0, pre_xt[2], out_pre[2])
            w_build_tile(7)
            pre_xt.append(emit_x_transpose(NPRE + 1))

            # ---------------- main loop: stream x ----------------
            for i in range(NTT):
                if i < NPRE:
                    nc.sync.dma_start(
                        out_ext[ts(i, 128), ts(0, 512)], out_pre[i][:, ts(0, 512)]
                    )
                    emit_mm_j(i, 1, pre_xt[i], out_pre[i])
                    nc.sync.dma_start(
                        out_ext[ts(i, 128), ts(1, 512)], out_pre[i][:, ts(1, 512)]
                    )
                elif i < NPRE + 2:
                    emit_x_mm(i, pre_xt[i])
                else:
                    emit_x_mm(i, emit_x_transpose(i))

    nc.compile()
    return nc


def kernel(x, W, C, U, R, bias):
    from concourse.bass_utils import run_bass_kernel_spmd

    x = np.ascontiguousarray(np.asarray(x, dtype=np.float32)).reshape(T, D)
    W = np.ascontiguousarray(np.asarray(W, dtype=np.float32))
    C = np.ascontiguousarray(np.asarray(C, dtype=np.float32))
    U = np.ascontiguousarray(np.asarray(U, dtype=np.float32))
    R = np.ascontiguousarray(np.asarray(R, dtype=np.float32))
    bias = np.ascontiguousarray(np.asarray(bias, dtype=np.float32))

    if "nc" not in _CACHE:
        _CACHE["nc"] = _build()
    nc = _CACHE["nc"]

    in_maps = []
    for core in range(N_CORES):
        tg, og = divmod(core, NO)
        in_maps.append(
            {
                "x": x[tg * TSH : (tg + 1) * TSH],
                "W": W[og * OSH : (og + 1) * OSH],
                "C": C[og * OSH : (og + 1) * OSH],
                "U": U,
                "R": R,
                "bias": bias[og * OSH : (og + 1) * OSH],
            }
        )

    res = run_bass_kernel_spmd(nc, in_maps, core_ids=list(range(N_CORES)))

    out = np.empty((T, O), dtype=np.float32)
    for core in range(N_CORES):
        tg, og = divmod(core, NO)
        out[tg * TSH : (tg + 1) * TSH, og * OSH : (og + 1) * OSH] = res.results[core][
            "out"
        ]
    return out.reshape(B, S, O)



# revision 2
# speedup vs baseline: 1.0756x; 1.0756x over previous
"""CURLoRA layer kernel for 8 TRN2 NeuronCores.

Computes out = x @ (W + C@U@R)^T + bias for
  x: (4, 2048, 4096) f32, W: (4096, 4096), C: (4096, 64), U: (64, 64),
  R: (64, 4096), bias: (4096,)  ->  out: (4, 2048, 4096) f32

Sharding: 8 cores = 2 token-groups x 4 output-column-groups.
Each core computes out[tg, og] = x[tg] @ (W[og] + C[og]@U@R)^T + bias[og]
independently (no collectives needed).

Host marshaling (layout/dtype only, all model FLOPs stay on device):
  x cast to bf16 (natural layout), W shard passed pre-transposed bf16
  W^T [D, OSH], C shard as C^T bf16 [64, OSH], U/R bf16, bias f32.

Per-core kernel (bf16 compute, fp32 accumulate):
  1. Plain-DMA W^T into resident SBUF wt_sb [128 dp, 32 kt, 1024 o] bf16.
  2. Adapter on PE: CU^T = U.T@C^T (K=64), then per kt-chunk
     (CUR)^T = R_chunk.T @ CU^T -> psum, DVE-added into wt_sb.
  3. x tiles loaded with DMA XBAR transpose straight from DRAM
     (bf16 [128t, 4096d] -> [128 dp, 32 kt, 128 t]) - the tensor engine
     does zero transposes.
  4. Main GEMM: per t-tile, 2 psum accumulations of 32 matmuls (N=512),
     bias added on eviction (DVE), out written f32.
"""

import sys

if "/opt/trn_rl_repo" not in sys.path:
    sys.path.insert(0, "/opt/trn_rl_repo")

import numpy as np
import ml_dtypes

BF16 = ml_dtypes.bfloat16

B, S, D = 4, 2048, 4096
O = 4096
RK = 64
T = B * S  # 8192 tokens
NT, NO = 2, 4  # token groups x out-column groups
TSH = T // NT  # 4096 tokens per core
OSH = O // NO  # 1024 out columns per core
N_CORES = 8

_CACHE = {}


def _build():
    from concourse import bacc
    import concourse.bass as bass
    import concourse.mybir as mybir
    from concourse.bass import ts
    from concourse.tile import TileContext

    f32 = mybir.dt.float32
    bf16 = mybir.dt.bfloat16

    nc = bacc.Bacc()
    x_ext = nc.declare_dram_parameter("x", [TSH, D], bf16, isOutput=False)
    wt_ext = nc.declare_dram_parameter("WT", [D, OSH], bf16, isOutput=False)
    ct_ext = nc.declare_dram_parameter("CT", [RK, OSH], bf16, isOutput=False)
    u_ext = nc.declare_dram_parameter("U", [RK, RK], bf16, isOutput=False)
    r_ext = nc.declare_dram_parameter("R", [RK, D], bf16, isOutput=False)
    b_ext = nc.declare_dram_parameter("bias", [OSH], f32, isOutput=False)
    out_ext = nc.declare_dram_parameter("out", [TSH, OSH], f32, isOutput=True)

    NKT = D // 128  # 32 k-tiles
    NTT = TSH // 128  # 32 t-tiles per core
    NOJ = OSH // 512  # 2 o-blocks of 512

    with TileContext(nc) as tc:
        with (
            tc.tile_pool(name="const", bufs=1) as const,
            tc.tile_pool(name="wt", bufs=1) as wtp,
            tc.tile_pool(name="small", bufs=1) as small,
            tc.tile_pool(name="xtpool", bufs=4) as xtpool,
            tc.tile_pool(name="opool", bufs=3) as opool,
            tc.tile_pool(name="psA", bufs=4, space="PSUM") as psA,
            tc.tile_pool(name="psB", bufs=3, space="PSUM") as psB,
        ):
            # --- small adapter inputs ---
            u_sb = small.tile([RK, RK], bf16)
            nc.scalar.dma_start(out=u_sb[:], in_=u_ext[:])
            ct_sb = small.tile([RK, OSH], bf16)
            nc.scalar.dma_start(out=ct_sb[:], in_=ct_ext[:])
            r_sb = small.tile([RK, D], bf16)
            nc.scalar.dma_start(out=r_sb[:], in_=r_ext[:])

            # bias broadcast to all 128 partitions
            bias_sb = const.tile([128, OSH], f32)
            b_ap = b_ext[:]
            b_bc = bass.AP(
                tensor=b_ap.tensor,
                offset=b_ap.offset,
                ap=[[0, 128]] + [list(p) for p in b_ap.ap],
            )
            nc.gpsimd.dma_start(out=bias_sb[:], in_=b_bc)

            # --- resident W^T: [128 dp, 32 kt, 1024 o] bf16 ---
            wt_sb = wtp.tile([128, NKT, OSH], bf16)
            wt_src = wt_ext[:].rearrange("(kt p) o -> p kt o", p=128)
            # chunked o-half-major so the adapter adds for j=0 can start early
            for jj in range(NOJ):
                for kh in range(2):
                    nc.scalar.dma_start(
                        out=wt_sb[:, ts(kh, NKT // 2), ts(jj, 512)],
                        in_=wt_src[:, ts(kh, NKT // 2), ts(jj, 512)],
                    )

            cut_sb = small.tile([RK, OSH], bf16)

            def emit_x_transpose(i):
                xT = xtpool.tile([128, NKT, 128], bf16)
                nc.sync.dma_start(
                    out=xT[:], in_=x_ext[ts(i, 128), :], transpose=True
                )
                return xT

            def emit_mm_j(i, j, xT, out_sb):
                psm = psA.tile([128, 512], f32, tag="a")
                for k in range(NKT):
                    nc.tensor.matmul(
                        psm[:],
                        xT[:, k, :],
                        wt_sb[:, k, ts(j, 512)],
                        start=(k == 0),
                        stop=(k == NKT - 1),
                    )
                nc.vector.tensor_add(
                    out=out_sb[:, ts(j, 512)],
                    in0=psm[:],
                    in1=bias_sb[:, ts(j, 512)],
                )

            # --- adapter: wt_sb += (C@U@R)^T, o-half-major ---
            for jj in range(NOJ):
                ps_cu = psB.tile([RK, 512], f32, tag="cu", bufs=1)
                nc.tensor.matmul(
                    ps_cu[:], u_sb[:], ct_sb[:, ts(jj, 512)], start=True, stop=True
                )
                nc.vector.tensor_copy(out=cut_sb[:, ts(jj, 512)], in_=ps_cu[:])
                for kt in range(NKT):
                    ps_ad = psB.tile([128, 512], f32, tag="ad", bufs=2)
                    nc.tensor.matmul(
                        ps_ad[:],
                        r_sb[:, ts(kt, 128)],
                        cut_sb[:, ts(jj, 512)],
                        start=True,
                        stop=True,
                    )
                    nc.vector.tensor_add(
                        out=wt_sb[:, kt, ts(jj, 512)],
                        in0=ps_ad[:],
                        in1=wt_sb[:, kt, ts(jj, 512)],
                    )

            # ---------------- main loop: stream x ----------------
            for i in range(NTT):
                xT = emit_x_transpose(i)
                out_sb = opool.tile([128, OSH], f32, tag="out")
                for j in range(NOJ):
                    emit_mm_j(i, j, xT, out_sb)
                    nc.gpsimd.dma_start(
                        out_ext[ts(i, 128), ts(j, 512)], out_sb[:, ts(j, 512)]
                    )

    nc.compile()
    return nc


def prepare_in_maps(x, W, C, U, R, bias):
    """Host-side marshaling: dtype casts + layout transposes + sharding."""
    x = np.asarray(x, dtype=np.float32).reshape(T, D).astype(BF16)
    W = np.asarray(W, dtype=np.float32)
    C = np.asarray(C, dtype=np.float32)
    U = np.ascontiguousarray(np.asarray(U, dtype=np.float32).astype(BF16))
    R = np.ascontiguousarray(np.asarray(R, dtype=np.float32).astype(BF16))
    bias = np.asarray(bias, dtype=np.float32)

    wt_sh = {}
    ct_sh = {}
    for og in range(NO):
        wt_sh[og] = np.ascontiguousarray(W[og * OSH : (og + 1) * OSH].T.astype(BF16))
        ct_sh[og] = np.ascontiguousarray(C[og * OSH : (og + 1) * OSH].T.astype(BF16))

    in_maps = []
    for core in range(N_CORES):
        tg, og = divmod(core, NO)
        in_maps.append(
            {
                "x": np.ascontiguousarray(x[tg * TSH : (tg + 1) * TSH]),
                "WT": wt_sh[og],
                "CT": ct_sh[og],
                "U": U,
                "R": R,
                "bias": np.ascontiguousarray(bias[og * OSH : (og + 1) * OSH]),
            }
        )
    return in_maps


def kernel(x, W, C, U, R, bias):
    from concourse.bass_utils import run_bass_kernel_spmd

    if "nc" not in _CACHE:
        _CACHE["nc"] = _build()
    nc = _CACHE["nc"]

    in_maps = prepare_in_maps(x, W, C, U, R, bias)
    res = run_bass_kernel_spmd(nc, in_maps, core_ids=list(range(N_CORES)))

    out = np.empty((T, O), dtype=np.float32)
    for core in range(N_CORES):
        tg, og = divmod(core, NO)
        out[tg * TSH : (tg + 1) * TSH, og * OSH : (og + 1) * OSH] = res.results[core][
            "out"
        ]
    return out.reshape(B, S, O)


# revision 5
# speedup vs baseline: 1.1388x; 1.0587x over previous
"""CURLoRA layer kernel for 8 TRN2 NeuronCores.

Computes out = x @ (W + C@U@R)^T + bias for
  x: (4, 2048, 4096) f32, W: (4096, 4096), C: (4096, 64), U: (64, 64),
  R: (64, 4096), bias: (4096,)  ->  out: (4, 2048, 4096) f32

Sharding: 8 cores = 2 token-groups x 4 output-column-groups.
Each core computes out[tg, og] = x[tg] @ (W[og] + C[og]@U@R)^T + bias[og]
independently (no collectives needed).

Host marshaling (layout/dtype only, all model FLOPs stay on device):
  x cast to bf16 (natural layout), W shard passed pre-transposed bf16
  W^T [D, OSH], C shard as C^T bf16 [64, OSH], U/R bf16, bias f32.

Per-core kernel (bf16 compute, fp32 accumulate):
  1. Plain-DMA W^T into resident SBUF wt_sb [128 dp, 32 kt, 1024 o] bf16.
  2. Adapter on PE: CU^T = U.T@C^T (K=64), then per kt-chunk
     (CUR)^T = R_chunk.T @ CU^T -> psum, DVE-added into wt_sb.
  3. x tiles loaded with DMA XBAR transpose straight from DRAM
     (bf16 [128t, 4096d] -> [128 dp, 32 kt, 128 t]) - the tensor engine
     does zero transposes.
  4. Main GEMM: per t-tile, 2 psum accumulations of 32 matmuls (N=512),
     bias added on eviction (DVE), out written f32.
"""

import sys

if "/opt/trn_rl_repo" not in sys.path:
    sys.path.insert(0, "/opt/trn_rl_repo")

import numpy as np
import ml_dtypes

BF16 = ml_dtypes.bfloat16

B, S, D = 4, 2048, 4096
O = 4096
RK = 64
T = B * S  # 8192 tokens
NT, NO = 2, 4  # token groups x out-column groups
TSH = T // NT  # 4096 tokens per core
OSH = O // NO  # 1024 out columns per core
N_CORES = 8

_CACHE = {}


def _build():
    from concourse import bacc
    import concourse.bass as bass
    import concourse.mybir as mybir
    from concourse.bass import ts
    from concourse.tile import TileContext
    from concourse.masks import make_identity

    f32 = mybir.dt.float32
    bf16 = mybir.dt.bfloat16

    nc = bacc.Bacc()
    x_ext = nc.declare_dram_parameter("x", [TSH, D], bf16, isOutput=False)
    wt_ext = nc.declare_dram_parameter("WT", [D, OSH], bf16, isOutput=False)
    ct_ext = nc.declare_dram_parameter("CT", [RK, OSH], bf16, isOutput=False)
    u_ext = nc.declare_dram_parameter("U", [RK, RK], bf16, isOutput=False)
    r_ext = nc.declare_dram_parameter("R", [RK, D], bf16, isOutput=False)
    b_ext = nc.declare_dram_parameter("bias", [OSH], f32, isOutput=False)
    out_ext = nc.declare_dram_parameter("out", [TSH, OSH], f32, isOutput=True)

    NKT = D // 128  # 32 k-tiles
    NTT = TSH // 128  # 32 t-tiles per core
    NOJ = OSH // 512  # 2 o-blocks of 512

    NPRE = 3  # t-tiles whose j=0 runs interleaved before the jj=1 adapter

    with TileContext(nc) as tc:
        with (
            tc.tile_pool(name="const", bufs=1) as const,
            tc.tile_pool(name="wt", bufs=1) as wtp,
            tc.tile_pool(name="small", bufs=1) as small,
            tc.tile_pool(name="xtpool", bufs=NPRE + 2) as xtpool,
            tc.tile_pool(name="opool", bufs=NPRE + 1) as opool,
            tc.tile_pool(name="psA", bufs=4, space="PSUM") as psA,
            tc.tile_pool(name="psB", bufs=3, space="PSUM") as psB,
        ):
            # identity for PE-warming dummy matmuls (HAM clock-gate: real
            # matmuls at t=0 bring the PE to 2.4GHz before real work lands)
            ident = const.tile([128, 128], bf16)
            make_identity(nc, ident)
            warm_ps = psB.tile([128, 128], f32, tag="warm", bufs=1)

            def emit_pe_warm(n):
                for _ in range(n):
                    nc.tensor.matmul(
                        warm_ps[:], ident[:], ident[:], start=True, stop=True
                    )

            # --- small adapter inputs first on the sync queue (XBARs queue
            # behind them); wt gets the scalar queue to itself ---
            u_sb = small.tile([RK, RK], bf16)
            nc.sync.dma_start(out=u_sb[:], in_=u_ext[:])
            ct_sb = small.tile([RK, OSH], bf16)
            nc.sync.dma_start(out=ct_sb[:], in_=ct_ext[:])
            r_sb = small.tile([RK, D], bf16)
            nc.sync.dma_start(out=r_sb[:], in_=r_ext[:])

            # bias broadcast to all 128 partitions
            bias_sb = const.tile([128, OSH], f32)
            b_ap = b_ext[:]
            b_bc = bass.AP(
                tensor=b_ap.tensor,
                offset=b_ap.offset,
                ap=[[0, 128]] + [list(p) for p in b_ap.ap],
            )
            nc.gpsimd.dma_start(out=bias_sb[:], in_=b_bc)

            emit_pe_warm(56)

            # --- resident W^T: [128 dp, 32 kt, 1024 o] bf16 ---
            wt_sb = wtp.tile([128, NKT, OSH], bf16)
            wt_src = wt_ext[:].rearrange("(kt p) o -> p kt o", p=128)
            # o-half-major, kt-quarter chunks: adds for jj=0 start early
            for jj in range(NOJ):
                for kh in range(4):
                    nc.scalar.dma_start(
                        out=wt_sb[:, ts(kh, NKT // 4), ts(jj, 512)],
                        in_=wt_src[:, ts(kh, NKT // 4), ts(jj, 512)],
                    )

            cut_sb = small.tile([RK, OSH], bf16)

            def emit_x_transpose(i):
                xT = xtpool.tile([128, NKT, 128], bf16)
                nc.sync.dma_start(
                    out=xT[:], in_=x_ext[ts(i, 128), :], transpose=True
                )
                return xT

            def emit_mm_j(i, j, xT, out_sb):
                psm = psA.tile([128, 512], f32, tag="a")
                for k in range(NKT):
                    nc.tensor.matmul(
                        psm[:],
                        xT[:, k, :],
                        wt_sb[:, k, ts(j, 512)],
                        start=(k == 0),
                        stop=(k == NKT - 1),
                    )
                nc.vector.tensor_add(
                    out=out_sb[:, ts(j, 512)],
                    in0=psm[:],
                    in1=bias_sb[:, ts(j, 512)],
                )

            def emit_adapter_half(jj):
                ps_cu = psB.tile([RK, 512], f32, tag="cu", bufs=1)
                nc.tensor.matmul(
                    ps_cu[:], u_sb[:], ct_sb[:, ts(jj, 512)], start=True, stop=True
                )
                nc.vector.tensor_copy(out=cut_sb[:, ts(jj, 512)], in_=ps_cu[:])
                for kt in range(NKT):
                    ps_ad = psB.tile([128, 512], f32, tag="ad", bufs=2)
                    nc.tensor.matmul(
                        ps_ad[:],
                        r_sb[:, ts(kt, 128)],
                        cut_sb[:, ts(jj, 512)],
                        start=True,
                        stop=True,
                    )
                    nc.vector.tensor_add(
                        out=wt_sb[:, kt, ts(jj, 512)],
                        in0=ps_ad[:],
                        in1=wt_sb[:, kt, ts(jj, 512)],
                    )

            # adapter jj=0, then first t-tiles' j=0 (so the PE has ready work
            # while wt jj=1 chunks + adapter jj=1 land), then adapter jj=1
            emit_adapter_half(0)

            pre_xt = [emit_x_transpose(i) for i in range(NPRE)]
            pre_out = [
                opool.tile([128, OSH], f32, tag="out", name="pre_out")
                for _ in range(NPRE)
            ]
            for i in range(NPRE):
                emit_mm_j(i, 0, pre_xt[i], pre_out[i])
                nc.gpsimd.dma_start(
                    out_ext[ts(i, 128), ts(0, 512)], pre_out[i][:, ts(0, 512)]
                )

            emit_adapter_half(1)

            # ---------------- main loop: stream x ----------------
            for i in range(NTT):
                if i < NPRE:
                    emit_mm_j(i, 1, pre_xt[i], pre_out[i])
                    nc.gpsimd.dma_start(
                        out_ext[ts(i, 128), ts(1, 512)], pre_out[i][:, ts(1, 512)]
                    )
                else:
                    xT = emit_x_transpose(i)
                    out_sb = opool.tile([128, OSH], f32, tag="out")
                    for j in range(NOJ):
                        emit_mm_j(i, j, xT, out_sb)
                        nc.gpsimd.dma_start(
                            out_ext[ts(i, 128), ts(j, 512)], out_sb[:, ts(j, 512)]
                        )

    nc.compile()
    return nc


def prepare_in_maps(x, W, C, U, R, bias):
    """Host-side marshaling: dtype casts + layout transposes + sharding."""
    x = np.asarray(x, dtype=np.float32).reshape(T, D).astype(BF16)
    W = np.asarray(W, dtype=np.float32)
    C = np.asarray(C, dtype=np.float32)
    U = np.ascontiguousarray(np.asarray(U, dtype=np.float32).astype(BF16))
    R = np.ascontiguousarray(np.asarray(R, dtype=np.float32).astype(BF16))
    bias = np.asarray(bias, dtype=np.float32)

    wt_sh = {}
    ct_sh = {}
    for og in range(NO):
        wt_sh[og] = np.ascontiguousarray(W[og * OSH : (og + 1) * OSH].T.astype(BF16))
        ct_sh[og] = np.ascontiguousarray(C[og * OSH : (og + 1) * OSH].T.astype(BF16))

    in_maps = []
    for core in range(N_CORES):
        tg, og = divmod(core, NO)
        in_maps.append(
            {
                "x": np.ascontiguousarray(x[tg * TSH : (tg + 1) * TSH]),
                "WT": wt_sh[og],
                "CT": ct_sh[og],
                "U": U,
                "R": R,
                "bias": np.ascontiguousarray(bias[og * OSH : (og + 1) * OSH]),
            }
        )
    return in_maps


def kernel(x, W, C, U, R, bias):
    from concourse.bass_utils import run_bass_kernel_spmd

    if "nc" not in _CACHE:
        _CACHE["nc"] = _build()
    nc = _CACHE["nc"]

    in_maps = prepare_in_maps(x, W, C, U, R, bias)
    try:
        res = run_bass_kernel_spmd(nc, in_maps, core_ids=list(range(N_CORES)))
    except Exception:
        # transient device hiccups (e.g. NRT exec-unit errors) usually clear
        # on a clean retry
        res = run_bass_kernel_spmd(nc, in_maps, core_ids=list(range(N_CORES)))

    out = np.empty((T, O), dtype=np.float32)
    for core in range(N_CORES):
        tg, og = divmod(core, NO)
        out[tg * TSH : (tg + 1) * TSH, og * OSH : (og + 1) * OSH] = res.results[core][
            "out"
        ]
    return out.reshape(B, S, O)
